# revision 32
# baseline (speedup 1.0000x reference)
"""Causal self-attention (B=4, T=2048, C=1024, H=16, D=64) on 8 TRN2 NeuronCores.

Sharding: 4 batches x 2 head-groups (8 heads each). Core c handles batch c//2,
heads 8*(c%2) .. 8*(c%2)+7. Host pre-transposes x and slices/transposes the
weights so the device kernel needs no on-chip transposes:

  phase 1:  qkT[feat, T] = Wqk_g @ x^T   (features on partitions, t-chunk outer
            with DMA admission chaining so the first matmul starts early)
            V[t, vfeat]  = x @ Wv_g^T    (keys on partitions, + ones column)
  phase 2:  per head-pair interleaved: S^T[k, q] = K_h Q_h^T (fp32r, k on
            partitions; the two heads of a pair use PE row groups 0-1/2-3
            concurrently), P^T = exp(S^T/8) * causal masks,
            out^T[d|sum, q] = [V_h|1]^T P^T, normalized by approx-reciprocal
            + gpsimd partition-broadcast of 1/sum
  phase 3:  y = attn^T.T @ Wp_g^T, interleaved between pairs as PE filler

Each core returns a [2048, 1024] partial; the host sums the two head-group
partials per batch.
"""

import numpy as np

T = 2048
N_CORES = 8

_CACHE = {}


def _build_module():
    from contextlib import ExitStack

    import concourse.tile as tile
    from concourse.tile_rust import add_dep_helper
    from concourse import bacc, mybir

    f32 = mybir.dt.float32
    f32r = mybir.dt.float32r
    Exp = mybir.ActivationFunctionType.Exp

    nc = bacc.Bacc("TRN2", target_bir_lowering=False, debug=False,
                   num_devices=N_CORES)

    xT_d = nc.dram_tensor("xT", (1024, 2048), f32r, kind="ExternalInput").ap()
    wqkT_d = nc.dram_tensor("wqkT", (1024, 1024), f32r, kind="ExternalInput").ap()
    wvT_d = nc.dram_tensor("wvT", (1024, 512), f32r, kind="ExternalInput").ap()
    wpT_d = nc.dram_tensor("wpT", (512, 1024), f32r, kind="ExternalInput").ap()
    mk_d = nc.dram_tensor("trimask", (128, 128), f32r, kind="ExternalInput").ap()
    y_d = nc.dram_tensor("y", (2048, 1024), f32, kind="ExternalOutput").ap()

    with tile.TileContext(nc) as tc, ExitStack() as ctx:
        pers = ctx.enter_context(tc.tile_pool(name="pers", bufs=1))
        sb_qT = pers.tile([128, 4, 2048], f32r, name="sb_qT")
        pq_tiles = [pers.tile([128, 512], f32r, name=f"pq{i}") for i in range(12)]
        zq = pers.tile([128, 512], f32, name="zq")
        sb_kT = pers.tile([128, 4, 2048], f32r, name="sb_kT")
        sb_v = pers.tile([128, 16, 520], f32r, name="sb_v")
        v_view = sb_v[:].rearrange("p t (h e) -> p t h e", e=65)

        nc.vector.memset(zq[:], 0.0)
        # pq0/1/4/5/8..11 serve even heads (zero band 64:128); pq2/3/6/7 odd
        PQ_EVEN = (0, 1, 4, 5, 8, 9, 10, 11)
        for i in range(12):
            o0, o1 = (64, 128) if i in PQ_EVEN else (0, 64)
            nc.vector.tensor_copy(pq_tiles[i][o0:o1, :], zq[o0:o1, :])

        # ---------------- phase 1: qkv projections ----------------
        with ExitStack() as p1:
            ph1 = p1.enter_context(tc.tile_pool(name="ph1", bufs=1))
            ps_big = p1.enter_context(tc.tile_pool(name="ps_big", bufs=4,
                                                   space="PSUM"))
            wqk_pool = p1.enter_context(tc.tile_pool(name="wqk", bufs=8))
            xt_pool = p1.enter_context(tc.tile_pool(name="xt", bufs=2))

            wqkT_r = wqkT_d.rearrange("(co ci) f -> ci co f", ci=128)
            xT_r = xT_d.rearrange("(co ci) t -> ci co t", ci=128)
            wvT_r = wvT_d.rearrange("(co ci) f -> ci co f", ci=128)

            # staged DMA admission: batch1 = wt4+wt5 + x chunk 0; later weights,
            # wvT and x chunks are chained so early-needed data transfers first.
            fbs = [4, 5, 6, 7, 0, 1, 2, 3]  # K features first, then Q
            wtiles = {}
            w_dmas = {}
            for fb in (4,):
                wt = wqk_pool.tile([128, 8, 128], f32r, tag="wqk", name=f"wt{fb}")
                w_dmas[fb] = nc.sync.dma_start(wt[:], wqkT_r[:, :, fb * 128:(fb + 1) * 128])
                wtiles[fb] = wt
            xchunk = xt_pool.tile([128, 8, 512], f32r, tag="xt", name="xc0")
            xdmas = [nc.sync.dma_start(xchunk[:, co, :], xT_r[:, co, 0:512])
                     for co in range(8)]
            batch = [(5, 6), (7, 0), (1, 2), (3,)]
            prev = xdmas[-1]
            for grp in batch:
                for fb in grp:
                    wt = wqk_pool.tile([128, 8, 128], f32r, tag="wqk", name=f"wt{fb}")
                    d = nc.sync.dma_start(wt[:], wqkT_r[:, :, fb * 128:(fb + 1) * 128])
                    add_dep_helper(d.ins, prev.ins, sync=False, reason="stage w")
                    wtiles[fb] = wt
                prev = d
            sb_wvT = ph1.tile([128, 8, 512], f32r, name="sb_wvT")
            for co in range(8):
                d = nc.sync.dma_start(sb_wvT[:, co, :], wvT_r[:, co, :])
                add_dep_helper(d.ins, prev.ins, sync=False, reason="stage wv")
            wv_last = d

            # ones column of sb_v via exp(0)=1 — also warms the ACT exp table
            zeros = ph1.tile([128, 128], f32, name="zeros")
            nc.vector.memset(zeros[:], 0.0)
            nc.scalar.activation(
                v_view[:, :, :, 64:65],
                zeros[:].rearrange("p (a b c) -> p a b c", a=16, b=8),
                Exp,
            )

            round_copy = {}
            prev_chunk_last = wv_last
            for tci in range(4):
                if tci < 3:
                    nxt = xt_pool.tile([128, 8, 512], f32r, tag="xt",
                                       name=f"xc{tci + 1}")
                    first = None
                    for co in range(8):
                        d = nc.sync.dma_start(
                            nxt[:, co, :],
                            xT_r[:, co, (tci + 1) * 512:(tci + 2) * 512])
                        if first is None:
                            add_dep_helper(d.ins, prev_chunk_last.ins,
                                           sync=False, reason="stage x chunk")
                            first = d
                    prev_chunk_last = d
                for fb in fbs:
                    dst, pblk = (sb_kT, fb - 4) if fb >= 4 else (sb_qT, fb)
                    ps = ps_big.tile([128, 512], f32, tag="psb")
                    for co in range(8):
                        nc.tensor.matmul(
                            ps[:],
                            lhsT=wtiles[fb][:, co, :],
                            rhs=xchunk[:, co, :],
                            start=(co == 0), stop=(co == 7),
                        )
                    if tci >= 2 and fb % 2 == 0:
                        cp = nc.scalar.activation(
                            dst[:, pblk, tci * 512:(tci + 1) * 512], ps[:],
                            mybir.ActivationFunctionType.Copy)
                    else:
                        cp = nc.vector.tensor_copy(
                            dst[:, pblk, tci * 512:(tci + 1) * 512], ps[:])
                    if fb == 4:
                        round_copy[tci] = cp
                    if tci == 0 and fb == 3:
                        # prefill padded-q for all of qc0
                        for pi, h in ((0, 0), (1, 2), (4, 4), (5, 6),
                                      (2, 1), (3, 3), (6, 5), (7, 7)):
                            rr0 = (h % 2) * 64
                            nc.vector.tensor_copy(
                                pq_tiles[pi][rr0:rr0 + 64, :],
                                sb_qT[rr0:rr0 + 64, h // 2, 0:512])
                    if tci == 1 and fb == 3:
                        # prefill qc1's even duos too (ramp relief)
                        for pi, h in ((8, 0), (9, 2), (10, 4), (11, 6)):
                            nc.vector.tensor_copy(
                                pq_tiles[pi][0:64, :],
                                sb_qT[0:64, h // 2, 512:1024])
                for tb in range(4):
                    tblk = tci * 4 + tb
                    ps = ps_big.tile([128, 512], f32, tag="psb")
                    for co in range(8):
                        nc.tensor.matmul(
                            ps[:],
                            lhsT=xchunk[:, co, tb * 128:(tb + 1) * 128],
                            rhs=sb_wvT[:, co, :],
                            start=(co == 0), stop=(co == 7),
                        )
                    from contextlib import nullcontext
                    lowprio = (tc.high_priority(offset=-400) if tci == 3
                               else nullcontext())
                    with lowprio:
                        if tci >= 2 and tb % 2 == 0:
                            nc.scalar.activation(
                                v_view[:, tblk, :, 0:64],
                                ps[:].rearrange("p (h d) -> p h d", d=64),
                                mybir.ActivationFunctionType.Copy)
                        else:
                            nc.vector.tensor_copy(
                                v_view[:, tblk, :, 0:64],
                                ps[:].rearrange("p (h d) -> p h d", d=64),
                            )
                if tci < 3:
                    xchunk = nxt

        # ---------------- phase 2 + 3: attention + proj ----------------
        ps_s = ctx.enter_context(tc.tile_pool(name="ps_s", bufs=2, space="PSUM"))
        ps_o = ctx.enter_context(tc.tile_pool(name="ps_o", bufs=4, space="PSUM"))
        ph2 = ctx.enter_context(tc.tile_pool(name="ph2", bufs=1))
        exp_pool = ctx.enter_context(tc.tile_pool(name="expp", bufs=3))
        norm_pool = ctx.enter_context(tc.tile_pool(name="normp", bufs=3))
        y_pool = ctx.enter_context(tc.tile_pool(name="yp", bufs=2))

        sb_attnT = ph2.tile([128, 4, 2048], f32r, name="sb_attnT")
        sb_wpT = ph2.tile([128, 4, 1024], f32r, name="sb_wpT")
        wpT_r = wpT_d.rearrange("(ko ki) n -> ki ko n", ki=128)
        for ko in range(4):
            dma = nc.sync.dma_start(sb_wpT[:, ko, :], wpT_r[:, ko, :])
            add_dep_helper(dma.ins, round_copy[2].ins, sync=False,
                           reason="admit wpT during round 2")
        sb_trimask = ph2.tile([128, 128], f32r, name="sb_trimask")
        dma = nc.sync.dma_start(sb_trimask[:], mk_d[:])
        add_dep_helper(dma.ins, round_copy[2].ins, sync=False,
                       reason="admit trimask during round 2")

        def emit_proj(tblk, on_act=False):
            for n in range(2):
                ysb = y_pool.tile([128, 512], f32, tag="ysb")
                pj = ps_o.tile([128, 512], f32, tag="pso")
                for ko in range(4):
                    nc.tensor.matmul(
                        pj[:],
                        lhsT=sb_attnT[:, ko, tblk * 128:(tblk + 1) * 128],
                        rhs=sb_wpT[:, ko, n * 512:(n + 1) * 512],
                        start=(ko == 0), stop=(ko == 3),
                    )
                if on_act:
                    nc.scalar.activation(ysb[:], pj[:],
                                         mybir.ActivationFunctionType.Copy)
                else:
                    nc.vector.tensor_copy(ysb[:], pj[:])
                nc.sync.dma_start(
                    y_d[tblk * 128:(tblk + 1) * 128, n * 512:(n + 1) * 512],
                    ysb[:])

        def norm_store(po, rr, p_, qc, on_act=False):
            att_slice = sb_attnT[rr:rr + 64, p_, qc * 512:(qc + 1) * 512]
            sums = norm_pool.tile([1, 512], f32, tag="sums")
            if on_act:
                nc.scalar.activation(att_slice, po[0:64, :],
                                     mybir.ActivationFunctionType.Copy)
                nc.scalar.activation(sums[:], po[64:65, :],
                                     mybir.ActivationFunctionType.Copy)
            else:
                nc.vector.tensor_copy(att_slice, po[0:64, :])
                nc.vector.tensor_copy(sums[:], po[64:65, :])
            recip = norm_pool.tile([1, 512], f32, tag="recip")
            nc.vector.reciprocal_approx_fast(out=recip[:], in_=sums[:])
            bcast = norm_pool.tile([128, 512], f32, tag="bcast")
            nc.gpsimd.partition_broadcast(bcast[:], recip[:])
            nc.vector.tensor_mul(att_slice, att_slice, bcast[rr:rr + 64, :])

        for qc in range(4):
            nblk = 4 * qc + 4
            # duo interleave: second head fills the first head's exp latency
            for di, (hA, hB) in enumerate(((0, 2), (4, 6), (1, 3), (5, 7))):
                rr = (hA % 2) * 64
                o0, o1 = (64, 128) if rr == 0 else (0, 64)
                poA = ps_o.tile([65, 512], f32, tag="pso")
                poB = ps_o.tile([65, 512], f32, tag="pso")
                if qc == 1 and di in (0, 1):
                    pqA, pqB = pq_tiles[8 + 2 * di], pq_tiles[9 + 2 * di]
                else:
                    base = (0, 4, 2, 6)[di]
                    pqA, pqB = pq_tiles[base], pq_tiles[base + 1]
                duo = ((hA, poA, pqA), (hB, poB, pqB))
                if qc > 0 and not (qc == 1 and di in (0, 1)):
                    for h, po, pq in duo:
                        nc.vector.tensor_copy(
                            pq[rr:rr + 64, :],
                            sb_qT[rr:rr + 64, h // 2, qc * 512:(qc + 1) * 512])
                for jg in range(nblk // 2):
                    # all scores of the duo first: the partner's scores hide
                    # each head's exp latency before its PV consumes it
                    ets = []
                    for h, po, pq in duo:
                        p_ = h // 2
                        pss = ps_s.tile([128, 2, 512], f32, tag="pss")
                        for jj in range(2):
                            j = jg * 2 + jj
                            nc.tensor.matmul(
                                pss[:, jj, :],
                                lhsT=sb_kT[:, p_, j * 128:(j + 1) * 128],
                                rhs=pq[:],
                                start=True, stop=True,
                            )
                        et = exp_pool.tile([128, 2, 512], f32r, tag="expT")
                        if jg * 2 == 4 * qc + 2:
                            # diagonal pair (i=2,3): exp only the live columns
                            nc.scalar.activation(et[:, 0, 256:512],
                                                 pss[:, 0, 256:512],
                                                 Exp, scale=0.125)
                            nc.scalar.activation(et[:, 1, 384:512],
                                                 pss[:, 1, 384:512],
                                                 Exp, scale=0.125)
                        else:
                            nc.scalar.activation(et[:], pss[:], Exp, scale=0.125)
                        ets.append(et)
                    for (h, po, pq), et in zip(duo, ets):
                        for jj in range(2):
                            j = jg * 2 + jj
                            lo = 0
                            if j >= 4 * qc:
                                i = j - 4 * qc
                                lo = i * 128
                                nc.vector.tensor_mul(
                                    et[:, jj, lo:lo + 128],
                                    et[:, jj, lo:lo + 128], sb_trimask[:])
                            nc.tensor.matmul(
                                po[:, lo:512],
                                lhsT=v_view[:, j, h, :],
                                rhs=et[:, jj, lo:512],
                                start=(j == 0), stop=(j == nblk - 1),
                            )
                for hi, (h, po, pq) in enumerate(duo):
                    norm_store(po, (h % 2) * 64, h // 2, qc,
                               on_act=(qc == 3 and di == 3 and hi == 1))
                if qc > 0:
                    emit_proj((qc - 1) * 4 + di)
            if qc == 3:
                for tblk in range(12, 16):
                    emit_proj(tblk, on_act=True)

    nc.compile()
    return nc


def _get_module():
    if "nc" not in _CACHE:
        _CACHE["nc"] = _build_module()
    return _CACHE["nc"]


def _make_trimask():
    # trimask[kk, q] = 1 iff q >= kk (diagonal 128x128 block)
    q = np.arange(128)[None, :]
    kk = np.arange(128)[:, None]
    return (q >= kk).astype(np.float32)


def make_in_maps(x, W_qkv, W_proj):
    x = np.asarray(x, dtype=np.float32)
    W_qkv = np.asarray(W_qkv, dtype=np.float32)
    W_proj = np.asarray(W_proj, dtype=np.float32)
    trimask = _make_trimask()
    in_maps = []
    for c in range(N_CORES):
        b, g = c // 2, c % 2
        s = 512 * g
        wqk = np.concatenate([W_qkv[s:s + 512], W_qkv[1024 + s:1024 + s + 512]], 0)
        in_maps.append({
            "xT": np.ascontiguousarray(x[b].T),
            "wqkT": np.ascontiguousarray(wqk.T),
            "wvT": np.ascontiguousarray(W_qkv[2048 + s:2048 + s + 512].T),
            "wpT": np.ascontiguousarray(W_proj[:, s:s + 512].T),
            "trimask": trimask,
        })
    return in_maps


def run(x, W_qkv, W_proj, trace=False):
    """Returns (y_full [4,2048,1024], BassKernelResults)."""
    from concourse import bass_utils

    nc = _get_module()
    in_maps = make_in_maps(x, W_qkv, W_proj)
    res = bass_utils.run_bass_kernel_spmd(
        nc, in_maps, core_ids=list(range(N_CORES)), trace=trace)
    y = np.zeros((4, T, 1024), np.float32)
    for b in range(4):
        y[b] = res.results[2 * b]["y"] + res.results[2 * b + 1]["y"]
    return y, res


def kernel(x, W_qkv, W_proj):
    y, _ = run(x, W_qkv, W_proj, trace=False)
    return y
